# revision 1
# baseline (speedup 1.0000x reference)
"""Trainium2 Bass kernel for nn_ConvNet_82978768159522 (GNN message passing).

Strategy (8 NeuronCores, SPMD):
  - Edges are sharded by dst-node range: core k owns nodes [k*1280, (k+1)*1280)
    and every edge whose dst lies in that range.  The segment_sum therefore
    needs no cross-core reduction; only the per-layer node-feature update is
    exchanged with an AllGather (x is replicated on every core).
  - Within a core, edges are grouped by 128-node "windows" so the
    gather(x[dst]) and the scatter-add become small one-hot matmuls against a
    window-resident table / PSUM accumulator.
  - Per-edge feature vectors live in "transposed" layout [U, e] so the U x U
    matmuls run with stationary weights and 512-wide moving operands.
  - x[src]-side tables (x@W2+b2 | x@Ws) are precomputed per layer into an HBM
    table and fetched per 512-edge group with a transposing dma_gather.
  - All matmul operands are bf16 (fp32 PSUM accumulation).
"""

import sys

for _p in ("/opt/trn_rl_repo",):
    if _p not in sys.path:
        sys.path.insert(0, _p)

import numpy as np
import ml_dtypes

import concourse.bass as bass
from concourse import bacc
import concourse.mybir as mybir
import concourse.tile as tile
from concourse import bass_utils

BF16 = mybir.dt.bfloat16
F32 = mybir.dt.float32
I16 = mybir.dt.int16
AF = mybir.ActivationFunctionType
ALU = mybir.AluOpType

NC_CORES = 8
U = 256  # hidden width (2 partition chunks of 128)
P = 128
EG = 512  # edges per group
WIN = 128  # nodes per scatter window


class Cfg:
    def __init__(self, N, E, L, NLOC, Gw):
        self.N, self.E, self.L = N, E, L
        self.NLOC = NLOC            # nodes owned per core (multiple of WIN)
        self.NPAD = NLOC * NC_CORES
        self.NWIN = NLOC // WIN
        self.Gw = Gw                # 512-edge groups per window
        self.G = self.NWIN * Gw     # groups per core
        self.EPAD = self.G * EG


def build_program(cfg: Cfg):
    nc = bacc.Bacc("TRN2", target_bir_lowering=False)
    L, G, Gw, NWIN, NPAD, NLOC = cfg.L, cfg.G, cfg.Gw, cfg.NWIN, cfg.NPAD, cfg.NLOC

    def inp(name, shape, dt):
        return nc.dram_tensor(name, shape, dt, kind="ExternalInput").ap()

    # ---- external inputs ----
    posT_g = inp("post_g", [2, NPAD], BF16)
    posT_l = inp("post_l", [2, NLOC], BF16)
    ea_g = inp("ea_g", [G, EG], BF16)
    idxs = inp("idxs", [P, (EG // 16) * G], I16)
    dst_col = inp("dst_col", [P, 4 * G], F32)
    iota_row = inp("iota_row", [P, P], F32)
    ident_bf = inp("ident_bf", [P, P], BF16)
    wp = inp("wp", [2, U], BF16)
    wa = inp("wa", [1, U], BF16)
    w1 = [inp(f"w1_{l}", [P, 2 * U], BF16) for l in range(L)]
    w2 = [inp(f"w2_{l}", [P, 2 * U], BF16) for l in range(L)]
    we = [inp(f"we_{l}", [P, 2 * U], BF16) for l in range(L + 1)]
    ws = [inp(f"ws_{l}", [P, 2 * U], BF16) for l in range(L + 1)]
    wt = [inp(f"wt_{l}", [P, 2 * U], BF16) for l in range(L + 1)]
    wm1a = inp("wm1a", [P, P], BF16)
    wm1b = inp("wm1b", [P, P], BF16)
    wm1c = inp("wm1c", [1, P], BF16)
    wm2 = inp("wm2", [P, 1], BF16)
    ones_bf = inp("ones_bf", [1, P], BF16)
    bp_col = inp("bp_col", [P, 2], F32)
    ba_col = inp("ba_col", [P, 2], F32)
    bm1_col = inp("bm1_col", [P, 1], F32)
    alpha_col = inp("alpha_col", [P, 1], F32)
    b2_rep = [inp(f"b2_rep_{l}", [P, U], F32) for l in range(L)]
    b1_row = [inp(f"b1_row_{l}", [1, U], BF16) for l in range(L)]
    be_row = [inp(f"be_row_{l}", [1, U], BF16) for l in range(L + 1)]

    out_d = nc.dram_tensor("out", [1, cfg.EPAD], F32, kind="ExternalOutput").ap()

    # ---- internal DRAM ----
    e_st = [nc.dram_tensor(f"e_st{i}", [G, U, EG], BF16).ap() for i in range(2)]
    gs_d = nc.dram_tensor("gs_d", [NPAD, 2 * U], BF16).ap()
    s3_d = nc.dram_tensor("s3_d", [NPAD, U], BF16).ap()
    ag_in = nc.dram_tensor("ag_in", [U, NLOC], BF16).ap()
    ag_out = nc.dram_tensor(
        "ag_out", [NC_CORES * U, NLOC], BF16, addr_space="Shared"
    ).ap()

    with tile.TileContext(nc) as tc:
        with (
            tc.tile_pool(name="const", bufs=1) as cp,
            tc.tile_pool(name="state", bufs=1) as sp,
            tc.tile_pool(name="io", bufs=3) as iop,
            tc.tile_pool(name="gath", bufs=3) as gp,
            tc.tile_pool(name="work", bufs=3) as wkp,
            tc.tile_pool(name="small", bufs=4) as smp,
            # PSUM budget is 8 banks of [128,512]xf32; tags share slots.
            tc.tile_pool(name="ps_main", bufs=3, space="PSUM") as pp_main,
            tc.tile_pool(name="ps_aux", bufs=2, space="PSUM") as pp_aux,
            tc.tile_pool(name="ps_msg", bufs=2, space="PSUM") as pp_msg,
            tc.tile_pool(name="ps_win", bufs=1, space="PSUM") as pp_win,
        ):
            r_eg = nc.gpsimd.to_reg(EG)

            # ---- load constants into SBUF ----
            def load_const(ap, shape, dt):
                t = cp.tile(shape, dt, tag=f"c{ap.tensor.name}")
                nc.sync.dma_start(out=t[:], in_=ap)
                return t

            c_idxs = load_const(idxs, [P, (EG // 16) * G], I16)
            c_dst = load_const(dst_col, [P, 4 * G], F32)
            c_iota = load_const(iota_row, [P, P], F32)
            c_id = load_const(ident_bf, [P, P], BF16)
            c_wp = load_const(wp, [2, U], BF16)
            c_wa = load_const(wa, [1, U], BF16)
            c_w1 = [load_const(w1[l], [P, 2 * U], BF16) for l in range(L)]
            c_w2 = [load_const(w2[l], [P, 2 * U], BF16) for l in range(L)]
            c_we = [load_const(we[l], [P, 2 * U], BF16) for l in range(L + 1)]
            c_ws = [load_const(ws[l], [P, 2 * U], BF16) for l in range(L + 1)]
            c_wt = [load_const(wt[l], [P, 2 * U], BF16) for l in range(L + 1)]
            c_wm1a = load_const(wm1a, [P, P], BF16)
            c_wm1b = load_const(wm1b, [P, P], BF16)
            c_wm1c = load_const(wm1c, [1, P], BF16)
            c_wm2 = load_const(wm2, [P, 1], BF16)
            c_ones = load_const(ones_bf, [1, P], BF16)
            c_bp = load_const(bp_col, [P, 2], F32)
            c_ba = load_const(ba_col, [P, 2], F32)
            c_bm1 = load_const(bm1_col, [P, 1], F32)
            c_alpha = load_const(alpha_col, [P, 1], F32)
            c_b2 = [load_const(b2_rep[l], [P, U], F32) for l in range(L)]
            c_b1 = [load_const(b1_row[l], [1, U], BF16) for l in range(L)]
            c_be = [load_const(be_row[l], [1, U], BF16) for l in range(L + 1)]
            c_posg = load_const(posT_g, [2, NPAD], BF16)
            c_posl = load_const(posT_l, [2, NLOC], BF16)

            # resident node state, transposed layout: [:, c*Ncols + n]
            xT_g = sp.tile([P, 2 * NPAD], BF16, tag="xT_g")
            xT_l = sp.tile([P, 2 * NLOC], BF16, tag="xT_l")

            # ---- x0 = pos @ Wp + bp  (built directly in T layout) ----
            def build_x0(dst_tile, src_pos, ncols):
                for c in range(2):
                    for s0 in range(0, ncols, EG):
                        sw = min(EG, ncols - s0)
                        ps = pp_aux.tile([P, EG], F32, tag="aux")
                        nc.tensor.matmul(
                            out=ps[:, :sw],
                            lhsT=c_wp[:2, c * P : (c + 1) * P],
                            rhs=src_pos[:2, s0 : s0 + sw],
                            start=True,
                            stop=True,
                        )
                        nc.scalar.activation(
                            out=dst_tile[:, c * ncols + s0 : c * ncols + s0 + sw],
                            in_=ps[:, :sw],
                            func=AF.Identity,
                            bias=c_bp[:, c : c + 1],
                        )

            build_x0(xT_g, c_posg, NPAD)
            build_x0(xT_l, c_posl, NLOC)

            # =================== layers ===================
            for l in range(L + 1):
                last = l == L
                # ---- node tables: gs = [x@W2+b2 | x@Ws]  (or s-only final) ----
                for s in range(NPAD // P):
                    xg0 = xT_g[:, s * P : (s + 1) * P]
                    xg1 = xT_g[:, NPAD + s * P : NPAD + (s + 1) * P]
                    tb_ps = pp_aux.tile([P, 2 * U], F32, tag="aux")
                    if not last:
                        nc.tensor.matmul(out=tb_ps[:, :U], lhsT=xg0,
                                         rhs=c_w2[l][:, :U], start=True, stop=False)
                        nc.tensor.matmul(out=tb_ps[:, :U], lhsT=xg1,
                                         rhs=c_w2[l][:, U:], start=False, stop=True)
                    nc.tensor.matmul(out=tb_ps[:, U:], lhsT=xg0,
                                     rhs=c_ws[l][:, :U], start=True, stop=False)
                    nc.tensor.matmul(out=tb_ps[:, U:], lhsT=xg1,
                                     rhs=c_ws[l][:, U:], start=False, stop=True)
                    if not last:
                        tb = wkp.tile([P, 2 * U], BF16, tag="tb")
                        nc.vector.tensor_add(
                            out=tb[:, :U], in0=tb_ps[:, :U], in1=c_b2[l][:]
                        )
                        nc.scalar.activation(
                            out=tb[:, U:], in_=tb_ps[:, U:], func=AF.Copy
                        )
                        nc.sync.dma_start(
                            out=gs_d[s * P : (s + 1) * P, :], in_=tb[:]
                        )
                    else:
                        tb = wkp.tile([P, U], BF16, tag="tb3")
                        nc.scalar.activation(
                            out=tb[:], in_=tb_ps[:, U:], func=AF.Copy
                        )
                        nc.sync.dma_start(
                            out=s3_d[s * P : (s + 1) * P, :], in_=tb[:]
                        )

                # ---- windows ----
                for w in range(NWIN):
                    xl0 = xT_l[:, w * P : (w + 1) * P]
                    xl1 = xT_l[:, NLOC + w * P : NLOC + (w + 1) * P]
                    # t_win = x_win @ Wt + be   [n, u']
                    tw_ps = pp_aux.tile([P, U], F32, tag="aux")
                    nc.tensor.matmul(out=tw_ps[:], lhsT=xl0, rhs=c_wt[l][:, :U],
                                     start=True, stop=False)
                    nc.tensor.matmul(out=tw_ps[:], lhsT=xl1, rhs=c_wt[l][:, U:],
                                     start=False, stop=False)
                    nc.tensor.matmul(out=tw_ps[:], lhsT=c_ones[:1, :],
                                     rhs=c_be[l][:1, :], start=False, stop=True)
                    t_win = smp.tile([P, U], BF16, tag="t_win")
                    nc.scalar.activation(out=t_win[:], in_=tw_ps[:], func=AF.Copy)

                    if not last:
                        # window accumulator: u1 + b1 (+ agg via scatter MMs)
                        pw = pp_win.tile([P, U], F32, tag="pw")
                        nc.tensor.matmul(out=pw[:], lhsT=xl0, rhs=c_w1[l][:, :U],
                                         start=True, stop=False)
                        nc.tensor.matmul(out=pw[:], lhsT=xl1, rhs=c_w1[l][:, U:],
                                         start=False, stop=False)
                        nc.tensor.matmul(out=pw[:], lhsT=c_ones[:1, :],
                                         rhs=c_b1[l][:1, :], start=False, stop=False)

                    for gw in range(Gw):
                        g = w * Gw + gw
                        lastg = gw == Gw - 1

                        # -- eT tiles [u-chunk][128, 512] --
                        eT = []
                        if l == 0:
                            ea_t = iop.tile([1, EG], BF16, tag="ea")
                            nc.sync.dma_start(out=ea_t[:], in_=ea_g[g : g + 1, :])
                            for c in range(2):
                                ps0 = pp_msg.tile([P, EG], F32, tag="msg")
                                nc.tensor.matmul(
                                    out=ps0[:],
                                    lhsT=c_wa[:1, c * P : (c + 1) * P],
                                    rhs=ea_t[:1, :],
                                    start=True, stop=True,
                                )
                                et = iop.tile([P, EG], BF16, tag=f"eT{c}")
                                nc.scalar.activation(
                                    out=et[:], in_=ps0[:], func=AF.Identity,
                                    bias=c_ba[:, c : c + 1],
                                )
                                eT.append(et)
                        else:
                            for c in range(2):
                                et = iop.tile([P, EG], BF16, tag=f"eT{c}")
                                nc.sync.dma_start(
                                    out=et[:],
                                    in_=e_st[(l - 1) % 2][g, c * P : (c + 1) * P, :],
                                )
                                eT.append(et)

                        # -- gather gs rows (transposing) --
                        nch = 2 if last else 4
                        gout = gp.tile([P, nch * EG], BF16, tag="gout")
                        gout3 = gout[:].rearrange("p (c e) -> p c e", c=nch)
                        nc.gpsimd.dma_gather(
                            out_ap=gout3,
                            in_ap=(s3_d if last else gs_d),
                            idxs_ap=c_idxs[:, g * (EG // 16) : (g + 1) * (EG // 16)],
                            num_idxs=EG,
                            num_idxs_reg=r_eg,
                            elem_size=(U if last else 2 * U),
                            transpose=True,
                        )
                        # gather chunk layout: [g0 g1 s0 s1] (or [s0 s1] final)
                        soff = 0 if last else 2

                        # -- one-hot masks (dst within window) --
                        oh = []
                        for q in range(4):
                            o = smp.tile([P, P], BF16, tag=f"oh{q}")
                            nc.vector.tensor_tensor(
                                out=o[:],
                                in0=c_dst[:, 4 * g + q : 4 * g + q + 1]
                                .to_broadcast([P, P]),
                                in1=c_iota[:],
                                op=ALU.is_equal,
                            )
                            oh.append(o)
                        ohT_ps = pp_msg.tile([P, EG], BF16, tag="msg")
                        for q in range(4):
                            nc.tensor.transpose(
                                out=ohT_ps[:, q * P : (q + 1) * P],
                                in_=oh[q][:], identity=c_id[:],
                            )
                        ohT = smp.tile([P, EG], BF16, tag="ohT")
                        nc.vector.tensor_copy(out=ohT[:], in_=ohT_ps[:])

                        if not last:
                            # -- msg = sigmoid(e) * g_src  (T layout) --
                            msgT = []
                            for c in range(2):
                                gate = wkp.tile([P, EG], BF16, tag=f"gate{c}")
                                nc.scalar.activation(
                                    out=gate[:], in_=eT[c][:], func=AF.Sigmoid
                                )
                                m = wkp.tile([P, EG], BF16, tag=f"msgT{c}")
                                nc.vector.tensor_mul(
                                    out=m[:], in0=gate[:],
                                    in1=gout3[:, c, :],
                                )
                                msgT.append(m)

                            # -- transpose msg to [e, u] and scatter-accumulate --
                            for q in range(4):
                                mq_ps = pp_msg.tile([P, U], BF16, tag="msg")
                                for c in range(2):
                                    nc.tensor.transpose(
                                        out=mq_ps[:, c * P : (c + 1) * P],
                                        in_=msgT[c][:, q * P : (q + 1) * P],
                                        identity=c_id[:],
                                    )
                                mq = wkp.tile([P, U], BF16, tag="mq")
                                nc.vector.tensor_copy(out=mq[:], in_=mq_ps[:])
                                nc.tensor.matmul(
                                    out=pw[:], lhsT=oh[q][:], rhs=mq[:],
                                    start=False,
                                    stop=(lastg and q == 3),
                                )

                        # -- pre-activation: We@e + t_dst(+be) [+ gathered s] --
                        enew = []
                        for c in range(2):
                            pe2 = pp_main.tile([P, EG], F32, tag="pe2")
                            nc.tensor.matmul(
                                out=pe2[:],
                                lhsT=c_we[l][:, c * P : (c + 1) * P],
                                rhs=eT[0][:], start=True, stop=False,
                            )
                            nc.tensor.matmul(
                                out=pe2[:],
                                lhsT=c_we[l][:, U + c * P : U + (c + 1) * P],
                                rhs=eT[1][:], start=False, stop=False,
                            )
                            nc.tensor.matmul(
                                out=pe2[:],
                                lhsT=t_win[:, c * P : (c + 1) * P],
                                rhs=ohT[:], start=False, stop=True,
                            )
                            z = wkp.tile([P, EG], BF16, tag=f"z{c}")
                            nc.vector.tensor_add(
                                out=z[:], in0=pe2[:], in1=gout3[:, soff + c, :]
                            )
                            en = iop.tile([P, EG], BF16, tag=f"en{c}")
                            # e_new = max(z, 0) + e
                            nc.vector.scalar_tensor_tensor(
                                out=en[:], in0=z[:], scalar=0.0, in1=eT[c][:],
                                op0=ALU.max, op1=ALU.add,
                            )
                            enew.append(en)
                            if not last:
                                nc.sync.dma_start(
                                    out=e_st[l % 2][g, c * P : (c + 1) * P, :],
                                    in_=en[:],
                                )

                        if last:
                            # -- fused MLP head --
                            ea_t = iop.tile([1, EG], BF16, tag="ea")
                            nc.sync.dma_start(out=ea_t[:], in_=ea_g[g : g + 1, :])
                            h_ps = pp_msg.tile([P, EG], F32, tag="msg")
                            nc.tensor.matmul(out=h_ps[:], lhsT=c_wm1a[:],
                                             rhs=enew[0][:], start=True, stop=False)
                            nc.tensor.matmul(out=h_ps[:], lhsT=c_wm1b[:],
                                             rhs=enew[1][:], start=False, stop=False)
                            nc.tensor.matmul(out=h_ps[:], lhsT=c_wm1c[:1, :],
                                             rhs=ea_t[:1, :], start=False, stop=True)
                            zp = wkp.tile([P, EG], BF16, tag="zp")
                            nc.vector.tensor_scalar_add(
                                out=zp[:], in0=h_ps[:], scalar1=c_bm1[:, :1]
                            )
                            h = wkp.tile([P, EG], BF16, tag="h")
                            # prelu: max(alpha*z, z)
                            nc.vector.scalar_tensor_tensor(
                                out=h[:], in0=zp[:], scalar=c_alpha[:, :1],
                                in1=zp[:], op0=ALU.mult, op1=ALU.max,
                            )
                            o_ps = pp_aux.tile([1, EG], F32, tag="aux")
                            nc.tensor.matmul(out=o_ps[:1, :], lhsT=c_wm2[:],
                                             rhs=h[:], start=True, stop=True)
                            o_sb = smp.tile([1, EG], F32, tag="o_sb")
                            nc.scalar.activation(out=o_sb[:1, :], in_=o_ps[:1, :],
                                                 func=AF.Copy)
                            nc.sync.dma_start(
                                out=out_d[:1, g * EG : (g + 1) * EG], in_=o_sb[:1, :]
                            )

                    if not last:
                        # ---- window x-update ----
                        xw_ps = pp_msg.tile([P, U], BF16, tag="msg")
                        nc.tensor.transpose(out=xw_ps[:, :P], in_=xl0,
                                            identity=c_id[:])
                        nc.tensor.transpose(out=xw_ps[:, P:], in_=xl1,
                                            identity=c_id[:])
                        xw = smp.tile([P, U], BF16, tag="xw")
                        nc.vector.tensor_copy(out=xw[:], in_=xw_ps[:])
                        xn = smp.tile([P, U], BF16, tag="xn")
                        # x_new = max(pw, 0) + x
                        nc.vector.scalar_tensor_tensor(
                            out=xn[:], in0=pw[:], scalar=0.0, in1=xw[:],
                            op0=ALU.max, op1=ALU.add,
                        )
                        xnT_ps = pp_msg.tile([P, U], BF16, tag="msg")
                        nc.tensor.transpose(out=xnT_ps[:, :P], in_=xn[:, :P],
                                            identity=c_id[:])
                        nc.tensor.transpose(out=xnT_ps[:, P:], in_=xn[:, P:],
                                            identity=c_id[:])
                        nc.vector.tensor_copy(out=xl0, in_=xnT_ps[:, :P])
                        nc.vector.tensor_copy(out=xl1, in_=xnT_ps[:, P:])

                if not last:
                    # ---- AllGather x ----
                    nc.sync.dma_start(out=ag_in[:P, :], in_=xT_l[:, :NLOC])
                    nc.sync.dma_start(out=ag_in[P:, :], in_=xT_l[:, NLOC:])
                    nc.gpsimd.collective_compute(
                        "AllGather",
                        ALU.bypass,
                        ins=[ag_in],
                        outs=[ag_out],
                        replica_groups=[list(range(NC_CORES))],
                    )
                    for c2 in range(NC_CORES):
                        for c in range(2):
                            nc.sync.dma_start(
                                out=xT_g[
                                    :, c * NPAD + c2 * NLOC : c * NPAD + (c2 + 1) * NLOC
                                ],
                                in_=ag_out[c2 * U + c * P : c2 * U + (c + 1) * P, :],
                            )

    nc.compile()
    return nc


# ======================= host side =======================

def host_prep(inputs, cfg: Cfg):
    """Shard + pack inputs for each core. Returns (in_maps, unperm)."""
    N, E, L = cfg.N, cfg.E, cfg.L
    NLOC, NPAD, NWIN, Gw, G, EPAD = (
        cfg.NLOC, cfg.NPAD, cfg.NWIN, cfg.Gw, cfg.G, cfg.EPAD)

    bf = ml_dtypes.bfloat16
    pos = np.asarray(inputs["pos"], np.float32)
    ea = np.asarray(inputs["edge_attr_in"], np.float32).reshape(-1)
    ei = np.asarray(inputs["edge_index"]).astype(np.int64)
    src, dst = ei[0], ei[1]

    pos_pad = np.zeros((NPAD, 2), np.float32)
    pos_pad[:N] = pos
    posT = np.ascontiguousarray(pos_pad.T).astype(bf)  # [2, NPAD]

    Wp = np.asarray(inputs["Wp"], np.float32)
    Wa = np.asarray(inputs["Wa"], np.float32)
    W1 = np.asarray(inputs["W1"], np.float32)
    W2 = np.asarray(inputs["W2"], np.float32)
    We = np.asarray(inputs["We"], np.float32)
    Ws = np.asarray(inputs["Ws"], np.float32)
    Wt = np.asarray(inputs["Wt"], np.float32)
    Wm1 = np.asarray(inputs["Wm1"], np.float32)
    Wm2 = np.asarray(inputs["Wm2"], np.float32)

    def wtile(W):  # [256,256] -> [128, 512] (k-chunks side by side)
        return np.concatenate([W[:P, :], W[P:, :]], axis=1).astype(bf)

    base = {
        "post_g": posT,
        "iota_row": np.tile(np.arange(P, dtype=np.float32)[None, :], (P, 1)),
        "ident_bf": np.eye(P, dtype=np.float32).astype(bf),
        "wp": Wp.astype(bf),
        "wa": Wa.astype(bf),
        "wm1a": Wm1[:P, :].astype(bf),
        "wm1b": Wm1[P : 2 * P, :].astype(bf),
        "wm1c": Wm1[2 * P : 2 * P + 1, :].astype(bf),
        "wm2": Wm2.astype(bf),
        "ones_bf": np.ones((1, P), np.float32).astype(bf),
        "bp_col": np.asarray(inputs["bp"], np.float32).reshape(2, P).T.copy(),
        "ba_col": np.asarray(inputs["ba"], np.float32).reshape(2, P).T.copy(),
        "bm1_col": np.asarray(inputs["bm1"], np.float32).reshape(P, 1).copy(),
        "alpha_col": np.full((P, 1), float(np.asarray(inputs["alpha"]).ravel()[0]),
                             np.float32),
    }
    for l in range(L):
        base[f"w1_{l}"] = wtile(W1[l])
        base[f"w2_{l}"] = wtile(W2[l])
        base[f"b2_rep_{l}"] = np.tile(
            np.asarray(inputs["b2"], np.float32)[l][None, :], (P, 1))
        base[f"b1_row_{l}"] = np.asarray(inputs["b1"], np.float32)[l][None, :].astype(bf)
    for l in range(L + 1):
        base[f"we_{l}"] = wtile(We[l])
        base[f"ws_{l}"] = wtile(Ws[l])
        base[f"wt_{l}"] = wtile(Wt[l])
        base[f"be_row_{l}"] = np.asarray(inputs["be"], np.float32)[l][None, :].astype(bf)

    in_maps = []
    unperm = []  # per core: original edge ids per slot (-1 = pad)
    for k in range(NC_CORES):
        m = dict(base)
        lo, hi = k * NLOC, (k + 1) * NLOC
        sel = np.nonzero((dst >= lo) & (dst < hi))[0]
        d_loc = dst[sel] - lo
        w_of = d_loc // WIN

        src_arr = np.zeros(EPAD, np.int64)
        dof_arr = np.full(EPAD, -1.0, np.float32)
        ea_arr = np.zeros(EPAD, np.float32)
        orig = np.full(EPAD, -1, np.int64)
        for w in range(NWIN):
            es = sel[w_of == w]
            n = len(es)
            assert n <= Gw * EG, f"window overflow: {n} > {Gw * EG}"
            b = w * Gw * EG
            src_arr[b : b + n] = src[es]
            dof_arr[b : b + n] = (dst[es] - lo - w * WIN).astype(np.float32)
            ea_arr[b : b + n] = ea[es]
            orig[b : b + n] = es

        idx16 = (
            src_arr.reshape(G, EG // 16, 16).transpose(0, 2, 1).reshape(G, 16, EG // 16)
        )
        # -> [16, G*(EG//16)] then tile to 128 partitions
        idx16 = np.concatenate([idx16[g] for g in range(G)], axis=1)
        m["idxs"] = np.tile(idx16, (8, 1)).astype(np.int16)
        m["dst_col"] = np.ascontiguousarray(
            dof_arr.reshape(G * 4, P).T)  # [128, 4G]
        m["ea_g"] = ea_arr.reshape(G, EG).astype(bf)
        m["post_l"] = np.ascontiguousarray(posT[:, lo:hi])
        in_maps.append(m)
        unperm.append(orig)

    return in_maps, unperm


_CACHE = {}


def _get_compiled(cfg_key, cfg):
    if cfg_key not in _CACHE:
        _CACHE[cfg_key] = build_program(cfg)
    return _CACHE[cfg_key]


def make_cfg(inputs):
    N, E, L = 10000, 320000, 3
    ei = np.asarray(inputs["edge_index"]).astype(np.int64)
    dst = ei[1]
    NLOC = 1280
    # groups per window: max window population, rounded up
    counts = np.bincount(dst // WIN, minlength=(NLOC * NC_CORES) // WIN)
    Gw = int(np.ceil(counts.max() / EG))
    return Cfg(N, E, L, NLOC, Gw)


def run(inputs, cfg, **spmd_kwargs):
    nc = _get_compiled((cfg.N, cfg.E, cfg.NLOC, cfg.Gw), cfg)
    in_maps, unperm = host_prep(inputs, cfg)
    res = bass_utils.run_bass_kernel_spmd(
        nc, in_maps, core_ids=list(range(NC_CORES)), **spmd_kwargs
    )
    out = np.zeros((cfg.E,), np.float32)
    for k in range(NC_CORES):
        o = np.asarray(res.results[k]["out"], np.float32).reshape(-1)
        mask = unperm[k] >= 0
        out[unperm[k][mask]] = o[mask]
    bm2 = float(np.asarray(inputs["bm2"]).ravel()[0])
    out = out + bm2
    return out[:, None].astype(np.float32), res


def kernel(**inputs) -> np.ndarray:
    out, _ = run(inputs, make_cfg(inputs))
    return out

